# revision 5
# baseline (speedup 1.0000x reference)
"""Trainium2 Bass kernel for BrainFunctionalConnectivityFeatureExtractionModule.

Math (per batch b, all f32):
    w    = relu(adj + adj_bias)                       (16,16)
    d    = 1/sqrt(sum(w, axis=1) + 1e-5)              (16,)
    lap  = I - d[:,None] * w * d[None,:]              (16,16)
    t1   = lap @ x[b]                                 (16,256)
    cp   = interleave(ones, t1)                       (16,512)
    h    = relu(brelu_bias + cp @ cheb_w)             (16,64)
    out  = h @ fc_w.T + fc_b                          (16,387)

Since the even interleaved lanes of cp are all-ones,
    cp @ cheb_w = t1 @ cheb_w[1::2] + sum(cheb_w[0::2], axis=0)
so the whole module collapses to three chained matmuls + relu:
    h   = relu(t1 @ W1 + bias_h),   W1 = cheb_w[1::2]  (256,64)
    out = h @ fc_w.T + fc_b

Device mapping (pure data parallel over 8 cores, B=8192 -> 1024 batches/core,
ROWS = 1024*16 = 16384 (b,e)-rows per core):

  The only awkward step is t1 = lap @ x[b]: the E-contraction happens along
  what is naturally the SBUF partition axis of x.  We fold the 16x16 lap mix
  into a "transposing" matmul:  with x loaded as [128 partitions, C] where
  partition p holds row l = 4p + s of a 512-row macro tile (s = 0..3 DMA
  sub-slot, giving contiguous 4KB DMA runs per partition), the matmul
      t1T[c, n] += x_chunk[:, s].T @ R_s       (accumulate over s)
  with R_s[p, b*16+i] = lap[i, (4p+s) % 16] * (b == (4p+s)//16)
  produces t1 TRANSPOSED: [C on partitions, 512 macro rows free] -- exactly
  the layout the W1 matmul needs.  All matmuls then run at full fp32r speed
  (N >= 256).  Stage 2: h^T[64, 512] = sum_k W1_k^T @ t1T_k.  Stage 3:
  out[128, 387] = (h^T slice).T @ fc_w^T, with the +fc_b fused into the
  PSUM->SBUF copy as a tensor_add against a partition-replicated fc_b tile.
"""

import numpy as np
from contextlib import ExitStack

B, E, C, H, OUT = 8192, 16, 256, 64, 387
NCORES = 8
ROWS = (B // NCORES) * E        # 16384 rows per core
NS = 4                          # DMA sub-slots per macro tile
TR = 128 * NS                   # 512 macro-tile rows
NT = ROWS // TR                 # 32 macro tiles per core
KC = C // 128                   # 2 contraction chunks of 128
OUTP = OUT + 1                  # fc matmul N padded even (fp32r ISA rule)

_cache = {}


def _build_module():
    import concourse.tile as tile
    from concourse import bacc, mybir

    f32 = mybir.dt.float32
    f32r = mybir.dt.float32r
    Relu = mybir.ActivationFunctionType.Relu

    nc = bacc.Bacc("TRN2", target_bir_lowering=False, debug=False,
                   num_devices=NCORES)

    x_d = nc.dram_tensor("x", (ROWS, C), f32r, kind="ExternalInput").ap()
    r_d = nc.dram_tensor("r", (NS, 128, TR), f32r, kind="ExternalInput").ap()
    w1_d = nc.dram_tensor("w1", (KC, 128, H), f32r, kind="ExternalInput").ap()
    bh_d = nc.dram_tensor("bh", (H, 1), f32, kind="ExternalInput").ap()
    fcw_d = nc.dram_tensor("fcw", (H, OUTP), f32r, kind="ExternalInput").ap()
    fcb_d = nc.dram_tensor("fcb", (128, OUT), f32, kind="ExternalInput").ap()
    o_d = nc.dram_tensor("o", (ROWS, OUT), f32, kind="ExternalOutput").ap()

    with tile.TileContext(nc) as tc:
        with ExitStack() as ctx:
            consts = ctx.enter_context(tc.tile_pool(name="consts", bufs=1))
            xp = ctx.enter_context(tc.tile_pool(name="xp", bufs=3))
            t1sp = ctx.enter_context(tc.tile_pool(name="t1sp", bufs=2))
            hp = ctx.enter_context(tc.tile_pool(name="hp", bufs=2))
            op = ctx.enter_context(tc.tile_pool(name="op", bufs=2))
            t1pp = ctx.enter_context(tc.tile_pool(name="t1pp", bufs=2, space="PSUM"))
            hpp = ctx.enter_context(tc.tile_pool(name="hpp", bufs=2, space="PSUM"))
            opp = ctx.enter_context(tc.tile_pool(name="opp", bufs=2, space="PSUM"))

            r_sb = consts.tile([128, NS, TR], f32r)
            nc.sync.dma_start(r_sb, r_d.rearrange("s p n -> p s n"))
            w1_sb = consts.tile([128, KC, H], f32r)
            nc.sync.dma_start(w1_sb, w1_d.rearrange("k p h -> p k h"))
            bh_sb = consts.tile([H, 1], f32)
            nc.sync.dma_start(bh_sb, bh_d)
            fcw_sb = consts.tile([H, OUTP], f32r)
            nc.sync.dma_start(fcw_sb, fcw_d)
            fcb_sb = consts.tile([128, OUT], f32)
            nc.sync.dma_start(fcb_sb, fcb_d)

            # row l of macro t lives at partition l//4, sub-slot l%4
            xv = x_d.rearrange("(t p s) c -> t p s c", p=128, s=NS)
            ov = o_d.rearrange("(t p s) o -> t p s o", p=128, s=NS)

            for t in range(NT):
                x_sb = xp.tile([128, NS, C], f32r)
                nc.sync.dma_start(x_sb, xv[t])

                # stage 1: t1T[c, n] = sum_s x[:, s, c_chunk].T @ R_s
                t1_ps = t1pp.tile([128, KC, TR], f32)
                for k in range(KC):
                    for s in range(NS):
                        nc.tensor.matmul(
                            t1_ps[:, k, :],
                            lhsT=x_sb[:, s, k * 128:(k + 1) * 128],
                            rhs=r_sb[:, s, :],
                            start=(s == 0),
                            stop=(s == NS - 1),
                        )
                t1_sb = t1sp.tile([128, KC, TR], f32r)
                nc.vector.tensor_copy(t1_sb[:, 0, :], t1_ps[:, 0, :])
                nc.scalar.copy(t1_sb[:, 1, :], t1_ps[:, 1, :])

                # stage 2: hT[h, n] = sum_k W1_k.T @ t1T_k
                h_ps = hpp.tile([H, TR], f32)
                for k in range(KC):
                    nc.tensor.matmul(
                        h_ps,
                        lhsT=w1_sb[:, k, :],
                        rhs=t1_sb[:, k, :],
                        start=(k == 0),
                        stop=(k == KC - 1),
                    )
                hT_sb = hp.tile([H, TR], f32r)
                nc.scalar.activation(hT_sb, h_ps, Relu, bias=bh_sb)

                # stage 3: out rows l = 4p + s -> sub-slot s takes hT cols s::4
                o_sb = op.tile([128, NS, OUT], f32)
                hT_v = hT_sb.rearrange("h (n s) -> h s n", s=NS)
                for s in range(NS):
                    o_ps = opp.tile([128, OUTP], f32)
                    nc.tensor.matmul(
                        o_ps,
                        lhsT=hT_v[:, s, :],
                        rhs=fcw_sb,
                    )
                    nc.vector.tensor_add(o_sb[:, s, :], o_ps[:, 0:OUT], fcb_sb)
                nc.sync.dma_start(ov[t], o_sb)

    nc.finalize()
    return nc


def _host_prep(adj, adj_bias, cheb_w, brelu_bias, fc_w, fc_b):
    adj = np.asarray(adj, np.float32)
    w = np.maximum(adj + np.float32(adj_bias.reshape(())), 0.0)
    d = 1.0 / np.sqrt(w.sum(axis=1) + np.float32(1e-5))
    lap = np.eye(E, dtype=np.float32) - d[:, None] * w * d[None, :]

    r = np.zeros((NS, 128, TR), np.float32)
    for s in range(NS):
        for p in range(128):
            l = NS * p + s
            b, j = divmod(l, E)
            r[s, p, b * E:(b + 1) * E] = lap[:, j]

    cheb_w = np.asarray(cheb_w, np.float32)
    w1 = np.ascontiguousarray(cheb_w[1::2, :]).reshape(KC, 128, H)
    bias_h = (cheb_w[0::2, :].sum(axis=0)
              + np.asarray(brelu_bias, np.float32).reshape(H))
    fcw = np.zeros((H, OUTP), np.float32)
    fcw[:, :OUT] = np.asarray(fc_w, np.float32).T
    fcb = np.ascontiguousarray(
        np.broadcast_to(np.asarray(fc_b, np.float32), (128, OUT)))
    return {
        "r": r,
        "w1": np.ascontiguousarray(w1),
        "bh": bias_h.reshape(H, 1).astype(np.float32),
        "fcw": fcw,
        "fcb": fcb,
    }


def _run(inputs, trace=False, **kw):
    from concourse import bass_utils

    if "nc" not in _cache:
        _cache["nc"] = _build_module()
    nc = _cache["nc"]

    x = np.asarray(inputs["x"], np.float32)
    weights = _host_prep(inputs["adj"], inputs["adj_bias"], inputs["cheb_w"],
                         inputs["brelu_bias"], inputs["fc_w"], inputs["fc_b"])

    shards = x.reshape(NCORES, ROWS, C)
    in_maps = [dict(weights, x=np.ascontiguousarray(shards[c]))
               for c in range(NCORES)]

    res = bass_utils.run_bass_kernel_spmd(
        nc, in_maps, core_ids=list(range(NCORES)), trace=trace, **kw)

    out = np.concatenate(
        [res.results[c]["o"].reshape(B // NCORES, E, OUT)
         for c in range(NCORES)], axis=0)
    return out, res


def kernel(**inputs) -> np.ndarray:
    out, _ = _run(inputs, trace=False)
    return out
